# revision 32
# baseline (speedup 1.0000x reference)
"""AngularLoss on 8 TRN2 NeuronCores (Bass/Tile), self-contained.

reference:
    emb = l2norm(embeddings); sim = emb @ emb.T; ang = acos(clip(sim, -1, 1))
    pos(i,p) = same-label & i!=p ; neg(i,n) = diff-label
    loss = sum over (i,p,n) [pos(i,p) & neg(i,n)] relu(ang[i,p]+a-ang[i,n]) / count

Distribution (SPMD, one NEFF): core c owns anchor chunk c//2 (128 rows)
and positive half c%2 (256 p's).  Per-core differences flow entirely
through the inputs: each core gets its own 128 anchor rows (emb_my) and a
*permuted* full embedding matrix (emb_pm) whose first 256 rows are that
core's p-half, so every slice in the program is static.

B^3 stage (default config BEST): one instruction per (p, chunk) tile
[128 anchors x 512 negatives]:
  - 206 p's on DVE: t = max(x_p, y_bf16) (tensor_scalar, bf16 packed reads)
    -> TensorE matmul ones.T @ t accumulates sum_{i,n} into PSUM [1,512];
    via relu(x-y) = max(x,y) - y, corrected by -n_dve * sum(y) at the end.
  - 50 p's on ACT: activation(Relu, scale=-1, bias=x_p, accum_out=rowsum)
    (accum_out = free-dim sum in the same instruction).
Masks are folded into the operands: x_p = (ang[i,p]+alpha)*pos (x=0 sentinel:
y>=0 makes terms vanish in both forms), y_n = ang[i,n] + 4*same (y>=4>max x
=> relu term vanishes; max-form contribution cancels exactly with the
correction since sum(y) is computed from the same bf16 values).

acos(s) = pi/2 - sign(s)*(pi/2 - 2*atan(w)), w = sqrt((1-|s|)/(1+|s|))
        = exp(0.5*(ln(1-|s|) - ln(1+|s|)))   [atan input stays in [0,1]]

Finale: [loss_partial, count/2] per core -> AllGather[8,2] -> sums -> divide.
"""

import math

import numpy as np

import concourse.bacc as bacc
import concourse.mybir as mybir
import concourse.tile as tile
from concourse.bass_utils import run_bass_kernel_spmd

B = 512
D = 512
N_CORES = 8
HALF = B // 2  # p's per core
ALPHA = math.radians(45.0)
CLIP = float(np.float32(1.0) - np.float32(2.0) ** -24)  # 0.99999994
PI_2 = math.pi / 2.0

Alu = mybir.AluOpType
Act = mybir.ActivationFunctionType
F32 = mybir.dt.float32
BF16 = mybir.dt.bfloat16
AX = mybir.AxisListType

N_DVE = 154  # main-loop tiles on DVE; rest on ACT
N_ACT = HALF - N_DVE


def _assign(n_dve, n_gps=0):
    """Evenly interleave engine assignment (d/a/g) for the 256 p-columns."""
    n_act = HALF - n_dve - n_gps
    errs = {"d": 0.0, "a": 0.0, "g": 0.0}
    rates = {"d": n_dve / HALF, "a": n_act / HALF, "g": n_gps / HALF}
    picks = []
    for _ in range(HALF):
        for e in ("d", "a", "g"):
            errs[e] += rates[e]
        best = max(errs, key=lambda e: errs[e])
        errs[best] -= 1.0
        picks.append(best)
    assert picks.count("d") == n_dve and picks.count("g") == n_gps
    return picks


def _body(nc, tc, emb_pm, lab_pm, iota_pm, emb_my, lab_my, iota_my, ident_d, out_d,
          reps=1, n_dve=N_DVE, skip_main=False, main_mode='mixed',
          dum_d_bf16=False, dum_a_psum=False, n_gps=0, y_bf16=False):
    picks = _assign(n_dve, n_gps)
    with (
        tc.tile_pool(name="persist", bufs=1) as sb,
        tc.tile_pool(name="work", bufs=2) as wk,
        tc.tile_pool(name="tp_ps", bufs=2, space="PSUM") as tp_ps,
        tc.tile_pool(name="bc_ps", bufs=2, space="PSUM") as bc_ps,
        tc.tile_pool(name="sim_ps", bufs=1, space="PSUM") as sim_ps,
        tc.tile_pool(name="fin_ps", bufs=1, space="PSUM") as fin_ps,
        tc.tile_pool(name="mn_ps", bufs=1, space="PSUM") as mn_ps,
        tc.tile_pool(name="dram", bufs=1, space="DRAM") as dram,
    ):
        # ---------------- load ----------------
        embp = [sb.tile([128, D], F32, tag=f"embp{k}", name=f"embp{k}") for k in range(4)]
        for k in range(4):
            nc.sync.dma_start(embp[k][:], emb_pm[128 * k : 128 * (k + 1), :])
        embm = sb.tile([128, D], F32, tag="embm")
        nc.sync.dma_start(embm[:], emb_my[:, :])
        labrow = sb.tile([1, B], F32, tag="labrow")
        nc.sync.dma_start(labrow[:], lab_pm.ap().rearrange("(a b) -> a b", a=1))
        iotarow = sb.tile([1, B], F32, tag="iotarow")
        nc.sync.dma_start(iotarow[:], iota_pm.ap().rearrange("(a b) -> a b", a=1))
        labm = sb.tile([128, 1], F32, tag="labm")
        nc.sync.dma_start(labm[:], lab_my[:, :])
        iotam = sb.tile([128, 1], F32, tag="iotam")
        nc.sync.dma_start(iotam[:], iota_my[:, :])
        ident = sb.tile([128, 128], F32, tag="ident")
        nc.sync.dma_start(ident[:], ident_d[:, :])

        ones1 = sb.tile([1, 128], F32, tag="ones1")
        nc.vector.memset(ones1[:], 1.0)
        ones8 = sb.tile([8, 1], F32, tag="ones8")
        nc.vector.memset(ones8[:], 1.0)
        ones128 = sb.tile([128, 1], F32, tag="ones128")
        nc.vector.memset(ones128[:], 1.0)
        ones128b = sb.tile([128, 1], BF16, tag="ones128b")
        nc.vector.memset(ones128b[:], 1.0)

        box = {}

        def compute():
            _compute(nc, tc, sb, wk, tp_ps, bc_ps, sim_ps, mn_ps, picks, box,
                     embp, embm, labrow, iotarow, labm, iotam, ident,
                     ones1, ones128, ones128b, n_dve, skip_main, main_mode,
                     dum_d_bf16, dum_a_psum, n_gps, y_bf16)

        if reps == 1:
            compute()
        else:
            with tc.For_i(0, reps, 1):
                compute()
        lc = box["lc"]

        # ---------------- partition reduce + AllGather + finale ----------
        part_ps = fin_ps.tile([1, 2], F32, tag="fin", name="part_ps")
        nc.tensor.matmul(part_ps[:], ones128[:], lc[:], start=True, stop=True)
        partial = sb.tile([1, 2], F32, tag="partial")
        nc.scalar.copy(partial[:], part_ps[:])
        for ex in box.get("extras", []):
            nc.vector.tensor_tensor(partial[:, 0:1], partial[:, 0:1],
                                    ex[:], Alu.add)

        cc_in = dram.tile([1, 2], F32, name="cc_in")
        cc_out = dram.tile([N_CORES, 2], F32, name="cc_out")
        nc.sync.dma_start(cc_in[:], partial[:])
        nc.gpsimd.collective_compute(
            "AllGather", Alu.bypass,
            replica_groups=[list(range(N_CORES))],
            ins=[cc_in[:].opt()], outs=[cc_out[:].opt()],
        )
        ag = sb.tile([N_CORES, 2], F32, tag="ag")
        nc.sync.dma_start(ag[:], cc_out[:])

        tot_ps = fin_ps.tile([1, 2], F32, tag="fin", name="tot_ps")
        nc.tensor.matmul(tot_ps[:], ones8[:], ag[:], start=True, stop=True)
        fin = sb.tile([1, 2], F32, tag="fin")
        nc.scalar.copy(fin[:], tot_ps[:])
        cclamp = sb.tile([1, 1], F32, tag="cclamp")
        nc.vector.tensor_scalar(cclamp[:], fin[:, 1:2], 1.0, None, Alu.max)
        crec = sb.tile([1, 1], F32, tag="crec")
        nc.vector.reciprocal(crec[:], cclamp[:])
        # gate on count > 0 (reference: where(count>0, loss/count, 0-valued loss))
        cgate = sb.tile([1, 1], F32, tag="cgate")
        nc.vector.tensor_scalar(cgate[:], fin[:, 1:2], 0.5, None, Alu.is_gt)
        crg = sb.tile([1, 1], F32, tag="crg")
        nc.vector.tensor_tensor(crg[:], crec[:], cgate[:], Alu.mult)
        res = sb.tile([1, 1], F32, tag="res")
        nc.vector.tensor_tensor(res[:], fin[:, 0:1], crg[:], Alu.mult)
        nc.sync.dma_start(out_d[:, :], res[:])


def _compute(nc, tc, sb, wk, tp_ps, bc_ps, sim_ps, mn_ps, picks, box,
             embp, embm, labrow, iotarow, labm, iotam, ident,
             ones1, ones128, ones128b, n_dve=N_DVE, skip_main=False,
             main_mode='mixed',
             dum_d_bf16=False, dum_a_psum=False, n_gps=0, y_bf16=False):
        n_act = HALF - n_dve - n_gps
        # ---------------- row norms (1/||row||) ----------------
        nsq = sb.tile([128, 5], F32, tag="nsq")
        for k in range(4):
            sqd = wk.tile([128, D], F32, tag="sqd")
            nc.scalar.activation(
                sqd[:], embp[k][:], Act.Square, accum_out=nsq[:, k : k + 1]
            )
        sqd = wk.tile([128, D], F32, tag="sqd")
        nc.scalar.activation(sqd[:], embm[:], Act.Square, accum_out=nsq[:, 4:5])
        nsqc = sb.tile([128, 5], F32, tag="nsqc")
        nc.vector.tensor_scalar(nsqc[:], nsq[:], 1e-24, None, Alu.max)
        lns = sb.tile([128, 5], F32, tag="lns")
        nc.scalar.activation(lns[:], nsqc[:], Act.Ln)
        rinv = sb.tile([128, 5], F32, tag="rinv")
        nc.scalar.activation(rinv[:], lns[:], Act.Exp, scale=-0.5)

        # normalize
        enp = [sb.tile([128, D], F32, tag=f"enp{k}", name=f"enp{k}") for k in range(4)]
        for k in range(4):
            nc.vector.tensor_scalar(
                enp[k][:], embp[k][:], rinv[:, k : k + 1], None, Alu.mult
            )
        enm = sb.tile([128, D], F32, tag="enm")
        nc.vector.tensor_scalar(enm[:], embm[:], rinv[:, 4:5], None, Alu.mult)

        # ---------------- transposes (PE) ----------------
        # enpT[k] [128d, 512j]: all permuted rows transposed; enmT[k] [128d,128r]
        enpT = [sb.tile([128, B], F32, tag=f"enpT{k}", name=f"enpT{k}") for k in range(4)]
        enmT = [sb.tile([128, 128], F32, tag=f"enmT{k}", name=f"enmT{k}") for k in range(4)]
        ncopy = 0
        for k in range(4):  # d-chunk
            for j in range(4):  # source row-chunk
                tp = tp_ps.tile([128, 128], F32, tag="tp")
                nc.tensor.transpose(tp[:], enp[j][:, 128 * k : 128 * (k + 1)], ident[:])
                if ncopy % 2 == 0:
                    nc.scalar.copy(enpT[k][:, 128 * j : 128 * (j + 1)], tp[:])
                else:
                    nc.vector.tensor_copy(enpT[k][:, 128 * j : 128 * (j + 1)], tp[:])
                ncopy += 1
            tp = tp_ps.tile([128, 128], F32, tag="tp")
            nc.tensor.transpose(tp[:], enm[:, 128 * k : 128 * (k + 1)], ident[:])
            if ncopy % 2 == 0:
                nc.scalar.copy(enmT[k][:], tp[:])
            else:
                nc.vector.tensor_copy(enmT[k][:], tp[:])
            ncopy += 1

        # ---------------- sim rows for my chunk ----------------
        simp = sim_ps.tile([128, B], F32, tag="simp")
        for k in range(4):
            nc.tensor.matmul(
                simp[:], enmT[k][:], enpT[k][:], start=(k == 0), stop=(k == 3)
            )

        # ---------------- acos ----------------
        s_c = sb.tile([128, B], F32, tag="s_c")  # clipped sim
        nc.vector.tensor_scalar(s_c[:], simp[:], CLIP, -CLIP, Alu.min, Alu.max)
        a_abs = sb.tile([128, B], F32, tag="a_abs")
        nc.vector.tensor_scalar(a_abs[:].bitcast(mybir.dt.int32),
                                s_c[:].bitcast(mybir.dt.int32),
                                0x7FFFFFFF, None, Alu.bitwise_and)
        ln1 = sb.tile([128, B], F32, tag="ln1")
        nc.scalar.activation(ln1[:], a_abs[:], Act.Ln, bias=1.0, scale=-1.0)  # ln(1-a)
        ln2 = sb.tile([128, B], F32, tag="ln2")
        nc.scalar.activation(ln2[:], a_abs[:], Act.Ln, bias=1.0, scale=1.0)  # ln(1+a)
        dln = sb.tile([128, B], F32, tag="dln")
        nc.vector.tensor_tensor(dln[:], ln1[:], ln2[:], Alu.subtract)
        w = sb.tile([128, B], F32, tag="w")
        nc.scalar.activation(w[:], dln[:], Act.Exp, scale=0.5)  # sqrt((1-a)/(1+a))
        at = sb.tile([128, B], F32, tag="at")
        nc.scalar.activation(at[:], w[:], Act.Arctan)
        sgb = sb.tile([128, B], F32, tag="sgb")  # sign bit of s
        nc.vector.tensor_scalar(sgb[:].bitcast(mybir.dt.int32),
                                s_c[:].bitcast(mybir.dt.int32),
                                -0x80000000, None, Alu.bitwise_and)
        u = sb.tile([128, B], F32, tag="u")  # pi/2 - 2*atan(w) >= 0
        nc.vector.tensor_scalar(u[:], at[:], -2.0, PI_2, Alu.mult, Alu.add)
        pr = sb.tile([128, B], F32, tag="pr")  # copysign(u, s)
        nc.vector.tensor_tensor(pr[:].bitcast(mybir.dt.int32),
                                u[:].bitcast(mybir.dt.int32),
                                sgb[:].bitcast(mybir.dt.int32), Alu.bitwise_or)
        ang = sb.tile([128, B], F32, tag="ang")  # acos = pi/2 - copysign(u, s)
        nc.vector.tensor_scalar(ang[:], pr[:], -1.0, PI_2, Alu.mult, Alu.add)

        # ---------------- masks ----------------
        labmat = bc_ps.tile([128, B], F32, tag="bcmat", name="labmat")
        nc.tensor.matmul(labmat[:], ones1[:], labrow[:], start=True, stop=True)
        iotamat = bc_ps.tile([128, B], F32, tag="bcmat", name="iotamat")
        nc.tensor.matmul(iotamat[:], ones1[:], iotarow[:], start=True, stop=True)

        sameS = sb.tile([128, B], F32, tag="sameS")  # 4.0 * same
        nc.vector.tensor_scalar(sameS[:], labmat[:], labm[:, 0:1], 4.0,
                                Alu.is_equal, Alu.mult)
        yneg = sb.tile([128, B], F32, tag="yneg")  # y = ang + 4*same
        nc.vector.tensor_tensor(yneg[:], ang[:], sameS[:], Alu.add)
        if y_bf16:
            yneg_bf = sb.tile([128, B], BF16, tag="yneg_bf")
            nc.vector.tensor_copy(yneg_bf[:], yneg[:])
            yneg_d = yneg_bf
        else:
            yneg_d = yneg

        ysum = sb.tile([128, 1], F32, tag="ysum")
        nc.vector.reduce_sum(out=ysum[:], in_=yneg_d[:], axis=AX.X)
        s4 = sb.tile([128, 1], F32, tag="s4")
        nc.vector.reduce_sum(out=s4[:], in_=sameS[:], axis=AX.X)

        # count_i = (s-1)*(B-s), s = s4/4 ; store 0.5*count (chunk shared by 2)
        t1 = sb.tile([128, 1], F32, tag="t1")
        nc.vector.tensor_scalar(t1[:], s4[:], 0.25, -1.0, Alu.mult, Alu.add)
        t2 = sb.tile([128, 1], F32, tag="t2")
        nc.vector.tensor_scalar(t2[:], s4[:], -0.25, float(B), Alu.mult, Alu.add)
        cnt = sb.tile([128, 1], F32, tag="cnt")
        nc.vector.tensor_tensor(cnt[:], t1[:], t2[:], Alu.mult)

        # X columns: x_p = (ang_p + alpha) * pos, p = permuted cols 0..255
        eyeP = sb.tile([128, HALF], F32, tag="eyeP")
        nc.vector.tensor_scalar(eyeP[:], iotamat[:, 0:HALF], iotam[:, 0:1], None,
                                Alu.is_equal)
        posP = sb.tile([128, HALF], F32, tag="posP")
        nc.vector.tensor_scalar(posP[:], sameS[:, 0:HALF], 0.25, None, Alu.mult)
        pos = sb.tile([128, HALF], F32, tag="pos")
        nc.vector.tensor_tensor(pos[:], posP[:], eyeP[:], Alu.subtract)
        angA = sb.tile([128, HALF], F32, tag="angA")
        nc.vector.tensor_scalar(angA[:], ang[:, 0:HALF], ALPHA, None, Alu.add)
        X = sb.tile([128, HALF], F32, tag="X")
        nc.vector.tensor_tensor(X[:], angA[:], pos[:], Alu.mult)

        # ---------------- B^3 main loop ----------------
        dacc = sb.tile([128, max(n_dve, 1)], F32, tag="dacc")
        aacc = sb.tile([128, max(n_act, 1)], F32, tag="aacc")
        dum_d = sb.tile([128, B], BF16 if (dum_d_bf16 or y_bf16) else F32,
                        tag="dum_d")
        if dum_a_psum:
            dum_a = mn_ps.tile([128, B], F32, tag="dum_a_ps")
        else:
            dum_a = sb.tile([128, B], F32, tag="dum_a")
        use_te = main_mode == "te" and not skip_main and n_dve > 0
        use_te2 = main_mode == "te2" and not skip_main
        dvacc_ps = (mn_ps.tile([1, B], F32, tag="dvacc_ps", name="dvacc_ps")
                    if use_te else None)
        if skip_main or n_dve == 0 or use_te or use_te2:
            nc.vector.memset(dacc[:], 0.0)
        if skip_main or n_act == 0 or use_te2:
            nc.vector.memset(aacc[:], 0.0)
        gacc = sb.tile([128, max(n_gps, 1)], F32, tag="gacc")
        nc.vector.memset(gacc[:], 0.0)
        use_gps = n_gps > 0 and not skip_main
        gacc_ps = mn_ps.tile([1, B], F32, tag="gacc_ps", name="gacc_ps") if use_gps else None
        if use_te2:
            dvacc_ps = mn_ps.tile([1, B], F32, tag="dvacc_ps", name="dvacc_ps")
        n_te_total = (n_dve + n_act) if use_te2 else (n_dve if use_te else 0)
        jt = 0
        jd = ja = jg = 0
        if not skip_main:
            for j in range(HALF):
                if picks[j] == "g":
                    tg = wk.tile([128, B], F32, tag="tg", name="tg", bufs=4)
                    nc.gpsimd.tensor_scalar(
                        tg[:], yneg[:], X[:, j : j + 1], None, Alu.max)
                    nc.tensor.matmul(gacc_ps[:], ones128[:], tg[:],
                                     start=(jg == 0), stop=(jg == n_gps - 1))
                    jg += 1
                elif picks[j] == "d":
                    if use_te or use_te2:
                        t = wk.tile([128, B], BF16 if y_bf16 else F32,
                                    tag="tmain", name="tmain", bufs=6)
                        nc.vector.tensor_scalar(
                            t[:], yneg_d[:], X[:, j : j + 1], None, Alu.max)
                        nc.tensor.matmul(dvacc_ps[:],
                                         ones128b[:] if y_bf16 else ones128[:],
                                         t[:], start=(jt == 0),
                                         stop=(jt == n_te_total - 1))
                        jt += 1
                    else:
                        nc.vector.tensor_scalar(
                            dum_d[:], yneg_d[:], X[:, j : j + 1], None,
                            Alu.max, Alu.add, accum_out=dacc[:, jd : jd + 1],
                        )
                    jd += 1
                else:
                    if use_te2:
                        ta = wk.tile([128, B], BF16 if y_bf16 else F32,
                                     tag="tact", name="tact", bufs=6)
                        nc.scalar.activation(
                            ta[:], yneg[:], Act.Relu, bias=X[:, j : j + 1],
                            scale=-1.0)
                        nc.tensor.matmul(dvacc_ps[:],
                                         ones128b[:] if y_bf16 else ones128[:],
                                         ta[:], start=(jt == 0),
                                         stop=(jt == n_te_total - 1))
                        jt += 1
                    else:
                        nc.scalar.activation(
                            dum_a[:], yneg[:], Act.Relu, bias=X[:, j : j + 1],
                            scale=-1.0, accum_out=aacc[:, ja : ja + 1],
                        )
                    ja += 1

        # ---------------- reduce + correction ----------------
        lsum_d = sb.tile([128, 1], F32, tag="lsum_d")
        nc.vector.reduce_sum(out=lsum_d[:], in_=dacc[:], axis=AX.X)
        lsum_a = sb.tile([128, 1], F32, tag="lsum_a")
        nc.vector.reduce_sum(out=lsum_a[:], in_=aacc[:], axis=AX.X)
        lsum_g = sb.tile([128, 1], F32, tag="lsum_g")
        nc.vector.reduce_sum(out=lsum_g[:], in_=gacc[:], axis=AX.X)
        n_corr = n_gps if not skip_main else 0
        if not (use_te or use_te2):
            n_corr += n_dve if not skip_main else 0
        corr = sb.tile([128, 1], F32, tag="corr")
        nc.vector.tensor_scalar(corr[:], ysum[:], -float(n_corr), None, Alu.mult)
        lsum = sb.tile([128, 1], F32, tag="lsum")
        nc.vector.tensor_tensor(lsum[:], lsum_d[:], lsum_a[:], Alu.add)
        lsum2 = sb.tile([128, 1], F32, tag="lsum2")
        nc.vector.tensor_tensor(lsum2[:], lsum[:], lsum_g[:], Alu.add)
        ltot = sb.tile([128, 1], F32, tag="ltot")
        nc.vector.tensor_tensor(ltot[:], lsum2[:], corr[:], Alu.add)

        # pack [loss_partial_col, 0.5*count_col, te_psum_row] for the finale
        lc = sb.tile([128, 2], F32, tag="lc")
        nc.vector.tensor_copy(lc[:, 0:1], ltot[:])
        nc.vector.tensor_scalar(lc[:, 1:2], cnt[:], 0.5, None, Alu.mult)
        box["lc"] = lc
        extras = []
        if use_te or use_te2:
            corr_ps = bc_ps.tile([1, B], F32, tag="corr_ps", name="corr_ps", bufs=1)
            nc.tensor.matmul(corr_ps[:],
                             ones128b[:] if y_bf16 else ones128[:],
                             yneg_d[:], start=True, stop=True)
            corr_row = sb.tile([1, B], F32, tag="corr_row")
            nc.scalar.copy(corr_row[:], corr_ps[:])
            dsum_row = sb.tile([1, B], F32, tag="dsum_row")
            nc.scalar.copy(dsum_row[:], dvacc_ps[:])
            corr_sc = sb.tile([1, B], F32, tag="corr_sc")
            nc.vector.tensor_scalar(corr_sc[:], corr_row[:], -float(n_dve), None,
                                    Alu.mult)
            comb_row = sb.tile([1, B], F32, tag="comb_row")
            nc.vector.tensor_tensor(comb_row[:], dsum_row[:], corr_sc[:], Alu.add)
            dsum_sc = sb.tile([1, 1], F32, tag="dsum_sc")
            nc.vector.reduce_sum(out=dsum_sc[:], in_=comb_row[:], axis=AX.X)
            extras.append(dsum_sc)
        if use_gps:
            gsum_row = sb.tile([1, B], F32, tag="gsum_row")
            nc.scalar.copy(gsum_row[:], gacc_ps[:])
            gsum_sc = sb.tile([1, 1], F32, tag="gsum_sc")
            nc.vector.reduce_sum(out=gsum_sc[:], in_=gsum_row[:], axis=AX.X)
            extras.append(gsum_sc)
        box["extras"] = extras


def _build(reps=1, n_dve=N_DVE, skip_main=False, main_mode='mixed',
           dum_d_bf16=False, dum_a_psum=False, n_gps=0, y_bf16=False):
    nc = bacc.Bacc(
        "TRN2", target_bir_lowering=False, debug=False, num_devices=N_CORES
    )
    emb_pm = nc.dram_tensor("emb_pm", [B, D], F32, kind="ExternalInput")
    lab_pm = nc.dram_tensor("lab_pm", [B], F32, kind="ExternalInput")
    iota_pm = nc.dram_tensor("iota_pm", [B], F32, kind="ExternalInput")
    emb_my = nc.dram_tensor("emb_my", [128, D], F32, kind="ExternalInput")
    lab_my = nc.dram_tensor("lab_my", [128, 1], F32, kind="ExternalInput")
    iota_my = nc.dram_tensor("iota_my", [128, 1], F32, kind="ExternalInput")
    ident_d = nc.dram_tensor("ident", [128, 128], F32, kind="ExternalInput")
    out_d = nc.dram_tensor("out", [1, 1], F32, kind="ExternalOutput")

    with tile.TileContext(nc) as tc:
        _body(nc, tc, emb_pm, lab_pm, iota_pm, emb_my, lab_my, iota_my,
              ident_d, out_d, reps=reps, n_dve=n_dve, skip_main=skip_main,
              main_mode=main_mode, dum_d_bf16=dum_d_bf16, dum_a_psum=dum_a_psum,
              n_gps=n_gps, y_bf16=y_bf16)
    nc.compile()
    return nc


_CACHE = {}


def make_in_maps(embeddings, labels):
    emb = np.ascontiguousarray(np.asarray(embeddings, dtype=np.float32))
    lab = np.asarray(labels).astype(np.float32)
    iota = np.arange(B, dtype=np.float32)
    ident = np.eye(128, dtype=np.float32)
    in_maps = []
    for c in range(N_CORES):
        chunk, half = c // 2, c % 2
        rows = slice(128 * chunk, 128 * (chunk + 1))
        pcols = np.arange(HALF * half, HALF * (half + 1))
        perm = np.concatenate([pcols, np.setdiff1d(np.arange(B), pcols)])
        in_maps.append({
            "emb_pm": np.ascontiguousarray(emb[perm]),
            "lab_pm": np.ascontiguousarray(lab[perm]),
            "iota_pm": np.ascontiguousarray(iota[perm]),
            "emb_my": np.ascontiguousarray(emb[rows]),
            "lab_my": np.ascontiguousarray(lab[rows]).reshape(128, 1),
            "iota_my": np.ascontiguousarray(iota[rows]).reshape(128, 1),
            "ident": ident,
        })
    return in_maps


BEST = dict(n_dve=206, main_mode="te", y_bf16=True)


def run(in_maps):
    nc = _CACHE.get("nc")
    if nc is None:
        nc = _build(**BEST)
        _CACHE["nc"] = nc
    res = run_bass_kernel_spmd(nc, in_maps, core_ids=list(range(N_CORES)))
    return res


def kernel(embeddings, labels):
    res = run(make_in_maps(embeddings, labels))
    val = np.float32(res.results[0]["out"][0, 0])
    return np.asarray(val, dtype=np.float32).reshape(())


# revision 33
# speedup vs baseline: 1.2513x; 1.2513x over previous
"""AngularLoss on 8 TRN2 NeuronCores (Bass/Tile), self-contained.

reference:
    emb = l2norm(embeddings); sim = emb @ emb.T; ang = acos(clip(sim, -1, 1))
    pos(i,p) = same-label & i!=p ; neg(i,n) = diff-label
    loss = sum over (i,p,n) [pos(i,p) & neg(i,n)] relu(ang[i,p]+a-ang[i,n]) / count

Distribution (SPMD, one NEFF): core c owns anchor chunk c//2 (128 rows)
and positive half c%2 (256 p's).  Per-core differences flow entirely
through the inputs: each core gets its own 128 anchor rows (emb_my) and a
*permuted* full embedding matrix (emb_pm) whose first 256 rows are that
core's p-half, so every slice in the program is static.

B^3 stage (default config BEST): one instruction per (p, chunk) tile
[128 anchors x 512 negatives]:
  - 206 p's on DVE: t = max(x_p, y_bf16) (tensor_scalar, bf16 packed reads)
    -> TensorE matmul ones.T @ t accumulates sum_{i,n} into PSUM [1,512];
    via relu(x-y) = max(x,y) - y, corrected by -n_dve * sum(y) at the end.
  - 50 p's on ACT: activation(Relu, scale=-1, bias=x_p, accum_out=rowsum)
    (accum_out = free-dim sum in the same instruction).
Masks are folded into the operands: x_p = (ang[i,p]+alpha)*pos (x=0 sentinel:
y>=0 makes terms vanish in both forms), y_n = ang[i,n] + 4*same (y>=4>max x
=> relu term vanishes; max-form contribution cancels exactly with the
correction since sum(y) is computed from the same bf16 values).

acos(s) = pi/2 - sign(s)*(pi/2 - 2*atan(w)), w = sqrt((1-|s|)/(1+|s|))
        = exp(0.5*(ln(1-|s|) - ln(1+|s|)))   [atan input stays in [0,1]]

Finale: [loss_partial, count/2] per core -> AllGather[8,2] -> sums -> divide.
"""

import math

import numpy as np

import concourse.bacc as bacc
import concourse.mybir as mybir
import concourse.tile as tile
from concourse.bass_utils import run_bass_kernel_spmd

B = 512
D = 512
N_CORES = 8
HALF = B // 2  # p's per core
ALPHA = math.radians(45.0)
CLIP = float(np.float32(1.0) - np.float32(2.0) ** -24)  # 0.99999994
PI_2 = math.pi / 2.0

Alu = mybir.AluOpType
Act = mybir.ActivationFunctionType
F32 = mybir.dt.float32
BF16 = mybir.dt.bfloat16
AX = mybir.AxisListType

N_DVE = 154  # main-loop tiles on DVE; rest on ACT
N_ACT = HALF - N_DVE


def _assign(n_dve, n_gps=0):
    """Evenly interleave engine assignment (d/a/g) for the 256 p-columns."""
    n_act = HALF - n_dve - n_gps
    errs = {"d": 0.0, "a": 0.0, "g": 0.0}
    rates = {"d": n_dve / HALF, "a": n_act / HALF, "g": n_gps / HALF}
    picks = []
    for _ in range(HALF):
        for e in ("d", "a", "g"):
            errs[e] += rates[e]
        best = max(errs, key=lambda e: errs[e])
        errs[best] -= 1.0
        picks.append(best)
    assert picks.count("d") == n_dve and picks.count("g") == n_gps
    return picks


def _body(nc, tc, emb_pm, lab_pm, iota_pm, emb_my, lab_my, iota_my, ident_d, out_d,
          reps=1, n_dve=N_DVE, skip_main=False, main_mode='mixed',
          dum_d_bf16=False, dum_a_psum=False, n_gps=0, y_bf16=False):
    picks = _assign(n_dve, n_gps)
    with (
        tc.tile_pool(name="persist", bufs=1) as sb,
        tc.tile_pool(name="work", bufs=2) as wk,
        tc.tile_pool(name="tp_ps", bufs=2, space="PSUM") as tp_ps,
        tc.tile_pool(name="bc_ps", bufs=2, space="PSUM") as bc_ps,
        tc.tile_pool(name="sim_ps", bufs=1, space="PSUM") as sim_ps,
        tc.tile_pool(name="fin_ps", bufs=1, space="PSUM") as fin_ps,
        tc.tile_pool(name="mn_ps", bufs=1, space="PSUM") as mn_ps,
        tc.tile_pool(name="dram", bufs=1, space="DRAM") as dram,
    ):
        # ---------------- load ----------------
        embp = [sb.tile([128, D], F32, tag=f"embp{k}", name=f"embp{k}") for k in range(4)]
        for k in range(4):
            nc.sync.dma_start(embp[k][:], emb_pm[128 * k : 128 * (k + 1), :])
        embm = sb.tile([128, D], F32, tag="embm")
        nc.sync.dma_start(embm[:], emb_my[:, :])
        labrow = sb.tile([1, B], F32, tag="labrow")
        nc.sync.dma_start(labrow[:], lab_pm.ap().rearrange("(a b) -> a b", a=1))
        iotarow = sb.tile([1, B], F32, tag="iotarow")
        nc.sync.dma_start(iotarow[:], iota_pm.ap().rearrange("(a b) -> a b", a=1))
        labm = sb.tile([128, 1], F32, tag="labm")
        nc.sync.dma_start(labm[:], lab_my[:, :])
        iotam = sb.tile([128, 1], F32, tag="iotam")
        nc.sync.dma_start(iotam[:], iota_my[:, :])
        ident = sb.tile([128, 128], F32, tag="ident")
        nc.sync.dma_start(ident[:], ident_d[:, :])

        ones1 = sb.tile([1, 128], F32, tag="ones1")
        nc.vector.memset(ones1[:], 1.0)
        ones8 = sb.tile([8, 1], F32, tag="ones8")
        nc.vector.memset(ones8[:], 1.0)
        ones128 = sb.tile([128, 1], F32, tag="ones128")
        nc.vector.memset(ones128[:], 1.0)
        ones128b = sb.tile([128, 1], BF16, tag="ones128b")
        nc.vector.memset(ones128b[:], 1.0)

        box = {}

        def compute():
            _compute(nc, tc, sb, wk, tp_ps, bc_ps, sim_ps, mn_ps, picks, box,
                     embp, embm, labrow, iotarow, labm, iotam, ident,
                     ones1, ones128, ones128b, n_dve, skip_main, main_mode,
                     dum_d_bf16, dum_a_psum, n_gps, y_bf16)

        if reps == 1:
            compute()
        else:
            with tc.For_i(0, reps, 1):
                compute()
        lc = box["lc"]

        # ---------------- partition reduce + AllGather + finale ----------
        part_ps = fin_ps.tile([1, 2], F32, tag="fin", name="part_ps")
        nc.tensor.matmul(part_ps[:], ones128[:], lc[:], start=True, stop=True)
        partial = sb.tile([1, 2], F32, tag="partial")
        nc.scalar.copy(partial[:], part_ps[:])
        for ex in box.get("extras", []):
            nc.vector.tensor_tensor(partial[:, 0:1], partial[:, 0:1],
                                    ex[:], Alu.add)

        cc_in = dram.tile([1, 2], F32, name="cc_in")
        cc_out = dram.tile([N_CORES, 2], F32, name="cc_out")
        nc.sync.dma_start(cc_in[:], partial[:])
        nc.gpsimd.collective_compute(
            "AllGather", Alu.bypass,
            replica_groups=[list(range(N_CORES))],
            ins=[cc_in[:].opt()], outs=[cc_out[:].opt()],
        )
        ag = sb.tile([N_CORES, 2], F32, tag="ag")
        nc.sync.dma_start(ag[:], cc_out[:])

        tot_ps = fin_ps.tile([1, 2], F32, tag="fin", name="tot_ps")
        nc.tensor.matmul(tot_ps[:], ones8[:], ag[:], start=True, stop=True)
        fin = sb.tile([1, 2], F32, tag="fin")
        nc.scalar.copy(fin[:], tot_ps[:])
        cclamp = sb.tile([1, 1], F32, tag="cclamp")
        nc.vector.tensor_scalar(cclamp[:], fin[:, 1:2], 1.0, None, Alu.max)
        crec = sb.tile([1, 1], F32, tag="crec")
        nc.vector.reciprocal(crec[:], cclamp[:])
        # gate on count > 0 (reference: where(count>0, loss/count, 0-valued loss))
        cgate = sb.tile([1, 1], F32, tag="cgate")
        nc.vector.tensor_scalar(cgate[:], fin[:, 1:2], 0.5, None, Alu.is_gt)
        crg = sb.tile([1, 1], F32, tag="crg")
        nc.vector.tensor_tensor(crg[:], crec[:], cgate[:], Alu.mult)
        res = sb.tile([1, 1], F32, tag="res")
        nc.vector.tensor_tensor(res[:], fin[:, 0:1], crg[:], Alu.mult)
        nc.sync.dma_start(out_d[:, :], res[:])


def _compute(nc, tc, sb, wk, tp_ps, bc_ps, sim_ps, mn_ps, picks, box,
             embp, embm, labrow, iotarow, labm, iotam, ident,
             ones1, ones128, ones128b, n_dve=N_DVE, skip_main=False,
             main_mode='mixed',
             dum_d_bf16=False, dum_a_psum=False, n_gps=0, y_bf16=False):
        n_act = HALF - n_dve - n_gps
        # ---------------- label-only masks (no emb dependency) ----------
        labmat = bc_ps.tile([128, B], F32, tag="bcmat", name="labmat")
        nc.tensor.matmul(labmat[:], ones1[:], labrow[:], start=True, stop=True)
        iotamat = bc_ps.tile([128, B], F32, tag="bcmat", name="iotamat")
        nc.tensor.matmul(iotamat[:], ones1[:], iotarow[:], start=True, stop=True)

        sameS = sb.tile([128, B], F32, tag="sameS")  # 4.0 * same
        nc.vector.tensor_scalar(sameS[:], labmat[:], labm[:, 0:1], 4.0,
                                Alu.is_equal, Alu.mult)
        eyeP = sb.tile([128, HALF], F32, tag="eyeP")
        nc.vector.tensor_scalar(eyeP[:], iotamat[:, 0:HALF], iotam[:, 0:1], None,
                                Alu.is_equal)
        posP = sb.tile([128, HALF], F32, tag="posP")
        nc.vector.tensor_scalar(posP[:], sameS[:, 0:HALF], 0.25, None, Alu.mult)
        pos = sb.tile([128, HALF], F32, tag="pos")
        nc.vector.tensor_tensor(pos[:], posP[:], eyeP[:], Alu.subtract)
        s4 = sb.tile([128, 1], F32, tag="s4")
        nc.vector.reduce_sum(out=s4[:], in_=sameS[:], axis=AX.X)
        t1 = sb.tile([128, 1], F32, tag="t1")
        nc.vector.tensor_scalar(t1[:], s4[:], 0.25, -1.0, Alu.mult, Alu.add)
        t2 = sb.tile([128, 1], F32, tag="t2")
        nc.vector.tensor_scalar(t2[:], s4[:], -0.25, float(B), Alu.mult, Alu.add)
        cnt = sb.tile([128, 1], F32, tag="cnt")
        nc.vector.tensor_tensor(cnt[:], t1[:], t2[:], Alu.mult)

        # ---------------- row norms (1/||row||) ----------------
        nsq = sb.tile([128, 5], F32, tag="nsq")
        for k in range(4):
            sqd = wk.tile([128, D], F32, tag="sqd")
            nc.scalar.activation(
                sqd[:], embp[k][:], Act.Square, accum_out=nsq[:, k : k + 1]
            )
        sqd = wk.tile([128, D], F32, tag="sqd")
        nc.scalar.activation(sqd[:], embm[:], Act.Square, accum_out=nsq[:, 4:5])
        nsqc = sb.tile([128, 5], F32, tag="nsqc")
        nc.vector.tensor_scalar(nsqc[:], nsq[:], 1e-24, None, Alu.max)
        lns = sb.tile([128, 5], F32, tag="lns")
        nc.scalar.activation(lns[:], nsqc[:], Act.Ln)
        rinv = sb.tile([128, 5], F32, tag="rinv")
        nc.scalar.activation(rinv[:], lns[:], Act.Exp, scale=-0.5)

        # normalize
        enp = [sb.tile([128, D], F32, tag=f"enp{k}", name=f"enp{k}") for k in range(4)]
        for k in range(4):
            nc.vector.tensor_scalar(
                enp[k][:], embp[k][:], rinv[:, k : k + 1], None, Alu.mult
            )
        enm = sb.tile([128, D], F32, tag="enm")
        nc.vector.tensor_scalar(enm[:], embm[:], rinv[:, 4:5], None, Alu.mult)

        # ---------------- transposes (PE) ----------------
        # enpT[k] [128d, 512j]: all permuted rows transposed; enmT[k] [128d,128r]
        enpT = [sb.tile([128, B], F32, tag=f"enpT{k}", name=f"enpT{k}") for k in range(4)]
        enmT = [sb.tile([128, 128], F32, tag=f"enmT{k}", name=f"enmT{k}") for k in range(4)]
        ncopy = 0
        for k in range(4):  # d-chunk
            for j in range(4):  # source row-chunk
                tp = tp_ps.tile([128, 128], F32, tag="tp")
                nc.tensor.transpose(tp[:], enp[j][:, 128 * k : 128 * (k + 1)], ident[:])
                if ncopy % 2 == 0:
                    nc.scalar.copy(enpT[k][:, 128 * j : 128 * (j + 1)], tp[:])
                else:
                    nc.vector.tensor_copy(enpT[k][:, 128 * j : 128 * (j + 1)], tp[:])
                ncopy += 1
            tp = tp_ps.tile([128, 128], F32, tag="tp")
            nc.tensor.transpose(tp[:], enm[:, 128 * k : 128 * (k + 1)], ident[:])
            if ncopy % 2 == 0:
                nc.scalar.copy(enmT[k][:], tp[:])
            else:
                nc.vector.tensor_copy(enmT[k][:], tp[:])
            ncopy += 1

        # ---------------- sim rows for my chunk ----------------
        simp = sim_ps.tile([128, B], F32, tag="simp")
        for k in range(4):
            nc.tensor.matmul(
                simp[:], enmT[k][:], enpT[k][:], start=(k == 0), stop=(k == 3)
            )

        # ---------------- acos ----------------
        s_c = sb.tile([128, B], F32, tag="s_c")  # clipped sim
        nc.vector.tensor_scalar(s_c[:], simp[:], CLIP, -CLIP, Alu.min, Alu.max)
        a_abs = sb.tile([128, B], F32, tag="a_abs")
        nc.vector.tensor_scalar(a_abs[:].bitcast(mybir.dt.int32),
                                s_c[:].bitcast(mybir.dt.int32),
                                0x7FFFFFFF, None, Alu.bitwise_and)
        ln1 = sb.tile([128, B], F32, tag="ln1")
        nc.scalar.activation(ln1[:], a_abs[:], Act.Ln, bias=1.0, scale=-1.0)  # ln(1-a)
        ln2 = sb.tile([128, B], F32, tag="ln2")
        nc.scalar.activation(ln2[:], a_abs[:], Act.Ln, bias=1.0, scale=1.0)  # ln(1+a)
        dln = sb.tile([128, B], F32, tag="dln")
        nc.vector.tensor_tensor(dln[:], ln1[:], ln2[:], Alu.subtract)
        w = sb.tile([128, B], F32, tag="w")
        nc.scalar.activation(w[:], dln[:], Act.Exp, scale=0.5)  # sqrt((1-a)/(1+a))
        at = sb.tile([128, B], F32, tag="at")
        nc.scalar.activation(at[:], w[:], Act.Arctan)
        sgb = sb.tile([128, B], F32, tag="sgb")  # sign bit of s
        nc.vector.tensor_scalar(sgb[:].bitcast(mybir.dt.int32),
                                s_c[:].bitcast(mybir.dt.int32),
                                -0x80000000, None, Alu.bitwise_and)
        u = sb.tile([128, B], F32, tag="u")  # pi/2 - 2*atan(w) >= 0
        nc.vector.tensor_scalar(u[:], at[:], -2.0, PI_2, Alu.mult, Alu.add)
        pr = sb.tile([128, B], F32, tag="pr")  # copysign(u, s)
        nc.vector.tensor_tensor(pr[:].bitcast(mybir.dt.int32),
                                u[:].bitcast(mybir.dt.int32),
                                sgb[:].bitcast(mybir.dt.int32), Alu.bitwise_or)
        ang = sb.tile([128, B], F32, tag="ang")  # acos = pi/2 - copysign(u, s)
        nc.vector.tensor_scalar(ang[:], pr[:], -1.0, PI_2, Alu.mult, Alu.add)

        # ---------------- ang-dependent operands ----------------
        yneg = sb.tile([128, B], F32, tag="yneg")  # y = ang + 4*same
        nc.vector.tensor_tensor(yneg[:], ang[:], sameS[:], Alu.add)
        if y_bf16:
            yneg_bf = sb.tile([128, B], BF16, tag="yneg_bf")
            nc.vector.tensor_copy(yneg_bf[:], yneg[:])
            yneg_d = yneg_bf
        else:
            yneg_d = yneg

        ysum = sb.tile([128, 1], F32, tag="ysum")
        nc.vector.reduce_sum(out=ysum[:], in_=yneg_d[:], axis=AX.X)

        # X columns: x_p = (ang_p + alpha) * pos, p = permuted cols 0..255
        angA = sb.tile([128, HALF], F32, tag="angA")
        nc.vector.tensor_scalar(angA[:], ang[:, 0:HALF], ALPHA, None, Alu.add)
        X = sb.tile([128, HALF], F32, tag="X")
        nc.vector.tensor_tensor(X[:], angA[:], pos[:], Alu.mult)

        # ---------------- B^3 main loop ----------------
        dacc = sb.tile([128, max(n_dve, 1)], F32, tag="dacc")
        aacc = sb.tile([128, max(n_act, 1)], F32, tag="aacc")
        dum_d = sb.tile([128, B], BF16 if (dum_d_bf16 or y_bf16) else F32,
                        tag="dum_d")
        if dum_a_psum:
            dum_a = mn_ps.tile([128, B], F32, tag="dum_a_ps")
        else:
            dum_a = sb.tile([128, B], F32, tag="dum_a")
        use_te = main_mode == "te" and not skip_main and n_dve > 0
        use_te2 = main_mode == "te2" and not skip_main
        dvacc_ps = (mn_ps.tile([1, B], F32, tag="dvacc_ps", name="dvacc_ps")
                    if use_te else None)
        if skip_main or n_dve == 0 or use_te or use_te2:
            nc.vector.memset(dacc[:], 0.0)
        if skip_main or n_act == 0 or use_te2:
            nc.vector.memset(aacc[:], 0.0)
        gacc = sb.tile([128, max(n_gps, 1)], F32, tag="gacc")
        nc.vector.memset(gacc[:], 0.0)
        use_gps = n_gps > 0 and not skip_main
        gacc_ps = mn_ps.tile([1, B], F32, tag="gacc_ps", name="gacc_ps") if use_gps else None
        if use_te2:
            dvacc_ps = mn_ps.tile([1, B], F32, tag="dvacc_ps", name="dvacc_ps")
        n_te_total = (n_dve + n_act) if use_te2 else (n_dve if use_te else 0)
        jt = 0
        jd = ja = jg = 0
        if not skip_main:
            for j in range(HALF):
                if picks[j] == "g":
                    tg = wk.tile([128, B], F32, tag="tg", name="tg", bufs=4)
                    nc.gpsimd.tensor_scalar(
                        tg[:], yneg[:], X[:, j : j + 1], None, Alu.max)
                    nc.tensor.matmul(gacc_ps[:], ones128[:], tg[:],
                                     start=(jg == 0), stop=(jg == n_gps - 1))
                    jg += 1
                elif picks[j] == "d":
                    if use_te or use_te2:
                        t = wk.tile([128, B], BF16 if y_bf16 else F32,
                                    tag="tmain", name="tmain", bufs=6)
                        nc.vector.tensor_scalar(
                            t[:], yneg_d[:], X[:, j : j + 1], None, Alu.max)
                        nc.tensor.matmul(dvacc_ps[:],
                                         ones128b[:] if y_bf16 else ones128[:],
                                         t[:], start=(jt == 0),
                                         stop=(jt == n_te_total - 1))
                        jt += 1
                    else:
                        nc.vector.tensor_scalar(
                            dum_d[:], yneg_d[:], X[:, j : j + 1], None,
                            Alu.max, Alu.add, accum_out=dacc[:, jd : jd + 1],
                        )
                    jd += 1
                else:
                    if use_te2:
                        ta = wk.tile([128, B], BF16 if y_bf16 else F32,
                                     tag="tact", name="tact", bufs=6)
                        nc.scalar.activation(
                            ta[:], yneg[:], Act.Relu, bias=X[:, j : j + 1],
                            scale=-1.0)
                        nc.tensor.matmul(dvacc_ps[:],
                                         ones128b[:] if y_bf16 else ones128[:],
                                         ta[:], start=(jt == 0),
                                         stop=(jt == n_te_total - 1))
                        jt += 1
                    else:
                        nc.scalar.activation(
                            dum_a[:], yneg[:], Act.Relu, bias=X[:, j : j + 1],
                            scale=-1.0, accum_out=aacc[:, ja : ja + 1],
                        )
                    ja += 1

        # ---------------- reduce + correction ----------------
        lsum_d = sb.tile([128, 1], F32, tag="lsum_d")
        nc.vector.reduce_sum(out=lsum_d[:], in_=dacc[:], axis=AX.X)
        lsum_a = sb.tile([128, 1], F32, tag="lsum_a")
        nc.vector.reduce_sum(out=lsum_a[:], in_=aacc[:], axis=AX.X)
        lsum_g = sb.tile([128, 1], F32, tag="lsum_g")
        nc.vector.reduce_sum(out=lsum_g[:], in_=gacc[:], axis=AX.X)
        n_corr = n_gps if not skip_main else 0
        if not (use_te or use_te2):
            n_corr += n_dve if not skip_main else 0
        corr = sb.tile([128, 1], F32, tag="corr")
        nc.vector.tensor_scalar(corr[:], ysum[:], -float(n_corr), None, Alu.mult)
        lsum = sb.tile([128, 1], F32, tag="lsum")
        nc.vector.tensor_tensor(lsum[:], lsum_d[:], lsum_a[:], Alu.add)
        lsum2 = sb.tile([128, 1], F32, tag="lsum2")
        nc.vector.tensor_tensor(lsum2[:], lsum[:], lsum_g[:], Alu.add)
        ltot = sb.tile([128, 1], F32, tag="ltot")
        nc.vector.tensor_tensor(ltot[:], lsum2[:], corr[:], Alu.add)

        # pack [loss_partial_col, 0.5*count_col, te_psum_row] for the finale
        lc = sb.tile([128, 2], F32, tag="lc")
        nc.vector.tensor_copy(lc[:, 0:1], ltot[:])
        nc.vector.tensor_scalar(lc[:, 1:2], cnt[:], 0.5, None, Alu.mult)
        box["lc"] = lc
        extras = []
        if use_te or use_te2:
            corr_ps = bc_ps.tile([1, B], F32, tag="corr_ps", name="corr_ps", bufs=1)
            nc.tensor.matmul(corr_ps[:],
                             ones128b[:] if y_bf16 else ones128[:],
                             yneg_d[:], start=True, stop=True)
            corr_row = sb.tile([1, B], F32, tag="corr_row")
            nc.scalar.copy(corr_row[:], corr_ps[:])
            dsum_row = sb.tile([1, B], F32, tag="dsum_row")
            nc.scalar.copy(dsum_row[:], dvacc_ps[:])
            corr_sc = sb.tile([1, B], F32, tag="corr_sc")
            nc.vector.tensor_scalar(corr_sc[:], corr_row[:], -float(n_dve), None,
                                    Alu.mult)
            comb_row = sb.tile([1, B], F32, tag="comb_row")
            nc.vector.tensor_tensor(comb_row[:], dsum_row[:], corr_sc[:], Alu.add)
            dsum_sc = sb.tile([1, 1], F32, tag="dsum_sc")
            nc.vector.reduce_sum(out=dsum_sc[:], in_=comb_row[:], axis=AX.X)
            extras.append(dsum_sc)
        if use_gps:
            gsum_row = sb.tile([1, B], F32, tag="gsum_row")
            nc.scalar.copy(gsum_row[:], gacc_ps[:])
            gsum_sc = sb.tile([1, 1], F32, tag="gsum_sc")
            nc.vector.reduce_sum(out=gsum_sc[:], in_=gsum_row[:], axis=AX.X)
            extras.append(gsum_sc)
        box["extras"] = extras


def _build(reps=1, n_dve=N_DVE, skip_main=False, main_mode='mixed',
           dum_d_bf16=False, dum_a_psum=False, n_gps=0, y_bf16=False):
    nc = bacc.Bacc(
        "TRN2", target_bir_lowering=False, debug=False, num_devices=N_CORES
    )
    emb_pm = nc.dram_tensor("emb_pm", [B, D], F32, kind="ExternalInput")
    lab_pm = nc.dram_tensor("lab_pm", [B], F32, kind="ExternalInput")
    iota_pm = nc.dram_tensor("iota_pm", [B], F32, kind="ExternalInput")
    emb_my = nc.dram_tensor("emb_my", [128, D], F32, kind="ExternalInput")
    lab_my = nc.dram_tensor("lab_my", [128, 1], F32, kind="ExternalInput")
    iota_my = nc.dram_tensor("iota_my", [128, 1], F32, kind="ExternalInput")
    ident_d = nc.dram_tensor("ident", [128, 128], F32, kind="ExternalInput")
    out_d = nc.dram_tensor("out", [1, 1], F32, kind="ExternalOutput")

    with tile.TileContext(nc) as tc:
        _body(nc, tc, emb_pm, lab_pm, iota_pm, emb_my, lab_my, iota_my,
              ident_d, out_d, reps=reps, n_dve=n_dve, skip_main=skip_main,
              main_mode=main_mode, dum_d_bf16=dum_d_bf16, dum_a_psum=dum_a_psum,
              n_gps=n_gps, y_bf16=y_bf16)
    nc.compile()
    return nc


_CACHE = {}


def make_in_maps(embeddings, labels):
    emb = np.ascontiguousarray(np.asarray(embeddings, dtype=np.float32))
    lab = np.asarray(labels).astype(np.float32)
    iota = np.arange(B, dtype=np.float32)
    ident = np.eye(128, dtype=np.float32)
    in_maps = []
    for c in range(N_CORES):
        chunk, half = c // 2, c % 2
        rows = slice(128 * chunk, 128 * (chunk + 1))
        pcols = np.arange(HALF * half, HALF * (half + 1))
        perm = np.concatenate([pcols, np.setdiff1d(np.arange(B), pcols)])
        in_maps.append({
            "emb_pm": np.ascontiguousarray(emb[perm]),
            "lab_pm": np.ascontiguousarray(lab[perm]),
            "iota_pm": np.ascontiguousarray(iota[perm]),
            "emb_my": np.ascontiguousarray(emb[rows]),
            "lab_my": np.ascontiguousarray(lab[rows]).reshape(128, 1),
            "iota_my": np.ascontiguousarray(iota[rows]).reshape(128, 1),
            "ident": ident,
        })
    return in_maps


BEST = dict(n_dve=206, main_mode="te", y_bf16=True)


def run(in_maps):
    nc = _CACHE.get("nc")
    if nc is None:
        nc = _build(**BEST)
        _CACHE["nc"] = nc
    res = run_bass_kernel_spmd(nc, in_maps, core_ids=list(range(N_CORES)))
    return res


def kernel(embeddings, labels):
    res = run(make_in_maps(embeddings, labels))
    val = np.float32(res.results[0]["out"][0, 0])
    return np.asarray(val, dtype=np.float32).reshape(())
